# revision 9
# baseline (speedup 1.0000x reference)
"""Sparsemax attention (B=2, H=16, L=S=2048, E=D=64, fp32) on 8 NeuronCores.

Strategy v5 (batch*head parallel, 4 (b,h) pairs per core):
  Three PE passes per unit (bh, 512-l-chunk), but the AV pass is *flipped*:
  A^T tiles (fp16, evicted by ACT/GPSIMD) are the stationary operand and V
  (fp16) is the moving operand, so AV costs 64 rows per matmul instead of
  512 -> PE drops from 164us to ~137us/core.

  Round 1 (per l-tile [128, S]): z = (Q K^T)/8 into PSUM; DVE top-8 per
    1024-half (exact; window schemes lose high-ranked support elements and
    blow the error gate).  tau is computed WITHOUT sorting the 16 candidates:
    with PA_i = prefix sums of the sorted h1-top8 and PB_j of h2-top8,
       tau = max_{i,j<=8} (PA_i + PB_j - 1) / (i+j)
    (every k<=16-prefix of the merged list is some (i,j) cell, and every other
    cell underestimates).  The 9x9 grid is built by GPSIMD TT-add/TT-mult on
    0-stride broadcast APs; DVE only does the two 8-wide prefix scans and one
    batched tensor_reduce(max) per unit.  DVE total ~168us/core = the wall.

  Round 2: z^T - tau via the 65th contraction row (khat row 64 = -1, qhat
    row 64 = tau, gathered by one SBUF->SBUF DMA per unit); Relu-evictions to
    fp16 att tiles split ACT(13)/GPSIMD(3) per unit -- DVE does none.

  AV: for lb in 0..3: accumulate over all 16 s-tiles into avp[:, lb*64:+64]
    ([128,256] f32 PSUM), lhsT = att[st][:, lb*128:+128] (fp16), rhs = vt[st]
    [128,64] fp16.  Output is [l, d] natural layout -> host just reshapes.

  Emission software-pipelines r1(u) / zt(u-ZT_D/4) / av(u-AV_D/4) at l-tile
  granularity as in v4.  All DMA on HWDGE queues (sync: tau gathers; scalar:
  bulk) -- the GPSIMD engine does evictions + grid work instead of SWDGE.
"""

import numpy as np

B, L, S, H, E, D = 2, 2048, 2048, 16, 64, 64
NCORES = 8
BHC = (B * H) // NCORES   # bh pairs per core = 4
NST = S // 128            # 16 s-tiles
NLC = L // 512            # 4 l-chunks

_nc = None


def _build():
    import os
    import concourse.bacc as bacc
    import concourse.mybir as mybir
    from concourse import tile

    F32 = mybir.dt.float32
    F32R = mybir.dt.float32r
    F16 = mybir.dt.float16
    AF = mybir.ActivationFunctionType
    OP = mybir.AluOpType
    AX = mybir.AxisListType

    nc = bacc.Bacc("TRN2", target_bir_lowering=False, debug=False)
    qt = nc.dram_tensor("qt", (BHC, E, L), F32R, kind="ExternalInput").ap()
    kh = nc.dram_tensor("kh", (BHC, E + 1, S), F32R, kind="ExternalInput").ap()
    vb = nc.dram_tensor("vb", (BHC, S, D), F16, kind="ExternalInput").ap()
    cst = nc.dram_tensor("cst", (128, 81), F32, kind="ExternalInput").ap()
    o = nc.dram_tensor("o", (BHC, L, D), F32, kind="ExternalOutput").ap()

    ZT_D = int(os.environ.get("ZT_D", "4"))
    AV_D = int(os.environ.get("AV_D", "8"))
    NGP = int(os.environ.get("NGP", "0"))     # DVE evicts per unit (of 16)

    with tile.TileContext(nc) as tc, \
         tc.tile_pool(name="const", bufs=1) as constp, \
         tc.tile_pool(name="big", bufs=4) as bigp, \
         tc.tile_pool(name="small", bufs=4) as smallp, \
         tc.tile_pool(name="att", bufs=44) as atp, \
         tc.tile_pool(name="outp", bufs=3) as outp, \
         tc.tile_pool(name="psA", bufs=2, space="PSUM") as psA, \
         tc.tile_pool(name="psAT", bufs=3, space="PSUM") as psAT, \
         tc.tile_pool(name="psAV", bufs=1, space="PSUM") as psAV:

        rec81 = constp.tile([128, 81], F32)

        tiles = {}

        def phase_a():
            # load order matters: HWDGE generates descriptors serially
            # (632ns/call), so K/Q chunks go first in unit-consumption order
            # (bh-major), then the grid constant, then V (needed only AV_D
            # segments in).
            for bh in range(BHC):
                qhat = bigp.tile([65, L], F32R, tag="qhat", name=f"qhat{bh}")
                khat = bigp.tile([65, S], F32R, tag="khat", name=f"khat{bh}")
                vtb = bigp.tile([128, NST * D], F16, tag="vtb", name=f"vtb{bh}")
                tiles[bh] = (qhat, khat, vtb)
            for bh in range(BHC):
                qhat, khat, _ = tiles[bh]
                if bh == 0:
                    nc.scalar.dma_start(out=khat[:, 0:512], in_=kh[0, :, 0:512])
                    nc.sync.dma_start(out=khat[:, 512:1024], in_=kh[0, :, 512:1024])
                    nc.scalar.dma_start(out=qhat[0:64, 0:512], in_=qt[0, :, 0:512])
                    nc.sync.dma_start(out=khat[:, 1024:2048], in_=kh[0, :, 1024:2048])
                    nc.scalar.dma_start(out=qhat[0:64, 512:1024],
                                        in_=qt[0, :, 512:1024])
                    nc.sync.dma_start(out=qhat[0:64, 1024:2048],
                                      in_=qt[0, :, 1024:2048])
                    nc.scalar.dma_start(out=rec81[:], in_=cst[:])
                    continue
                qq = nc.sync if bh % 2 else nc.scalar
                for pos in (0, 1024):
                    nc.scalar.dma_start(out=khat[:, pos:pos + 1024],
                                        in_=kh[bh, :, pos:pos + 1024])
                    qq.dma_start(out=qhat[0:64, pos:pos + 1024],
                                 in_=qt[bh, :, pos:pos + 1024])
            for bh in range(BHC):
                nc.scalar.dma_start(
                    out=tiles[bh][2][:].rearrange("p (st d) -> p st d", st=NST),
                    in_=vb[bh].rearrange("(st p) d -> p st d", p=128))

        def emit_r1_tile(bh, lc, ii, gt):
            """Round 1 for one l-tile: z, top8 per half, prefix scans, grid."""
            qhat, khat, _ = tiles[bh]
            i = lc * 4 + ii
            cands = smallp.tile([128, 16], F32, tag="cands", name=f"cd{bh}{lc}{ii}")
            for c in range(2):
                ps = psA.tile([128, 1024], F32, tag="r1", name=f"ps{bh}{lc}{ii}{c}")
                for half in range(2):
                    nc.tensor.matmul(
                        ps[:, half * 512:(half + 1) * 512],
                        lhsT=qhat[0:64, i * 128:(i + 1) * 128],
                        rhs=khat[0:64, c * 1024 + half * 512:
                                 c * 1024 + (half + 1) * 512],
                        start=True, stop=True)
                nc.vector.max(out=cands[:, c * 8:(c + 1) * 8], in_=ps[:])
            pa = smallp.tile([128, 9], F32, tag="pa", name=f"pa{bh}{lc}{ii}")
            pb = smallp.tile([128, 9], F32, tag="pb", name=f"pb{bh}{lc}{ii}")
            nc.gpsimd.memset(pa[:, 0:1], -1.0)
            nc.gpsimd.memset(pb[:, 0:1], 0.0)
            nc.vector.tensor_tensor_scan(
                out=pa[:, 1:9], data0=cands[:, 0:8], data1=cands[:, 0:8],
                initial=-1.0, op0=OP.add, op1=OP.bypass)
            nc.vector.tensor_tensor_scan(
                out=pb[:, 1:9], data0=cands[:, 8:16], data1=cands[:, 8:16],
                initial=0.0, op0=OP.add, op1=OP.bypass)
            gadd = smallp.tile([128, 81], F32, tag="gadd", name=f"ga{bh}{lc}{ii}")
            nc.gpsimd.tensor_tensor(
                out=gadd[:].rearrange("p (i j) -> p i j", i=9),
                in0=pa[:].to_broadcast((128, 9, 9)),
                in1=pb[:].to_broadcast((128, 9, 9)).rearrange("p j i -> p i j"),
                op=OP.add)
            nc.gpsimd.tensor_tensor(
                out=gt[:, ii * 81:(ii + 1) * 81], in0=gadd[:], in1=rec81[:],
                op=OP.mult)

        def emit_r1_tail(bh, lc, gt):
            # batched tau for the 4 l-tiles + one SBUF->SBUF gather DMA into
            # qhat row 64 (the tau contraction row for round 2)
            tau4 = smallp.tile([128, 4], F32R, tag="tau4", name=f"t4{bh}{lc}")
            nc.vector.tensor_reduce(
                out=tau4[:], in_=gt[:].rearrange("p (lt c) -> p lt c", lt=4),
                axis=AX.X, op=OP.max)
            qhat = tiles[bh][0]
            for jj in range(4):
                nc.sync.dma_start(
                    out=qhat[64:65, lc * 512 + jj * 128:lc * 512 + (jj + 1) * 128],
                    in_=tau4[:, jj:jj + 1])

        atts = {}

        def emit_zt_evict(bh, lc, st0, n_st, dve_assist=False):
            """z^T - tau for s-tiles st0..+n_st, relu-evicted to fp16 SBUF.
            dve_assist: drain region -- DVE (idle there) takes every other
            eviction so the tail isn't serialized on ACT."""
            qhat, khat, _ = tiles[bh]
            for st in range(st0, st0 + n_st):
                atps = psAT.tile([128, 512], F32, tag="at", name=f"at{bh}{lc}{st}")
                att = atp.tile([128, 512], F16, tag="att", name=f"a{bh}{lc}{st}")
                nc.tensor.matmul(atps[:], lhsT=khat[:, st * 128:(st + 1) * 128],
                                 rhs=qhat[:, lc * 512:(lc + 1) * 512],
                                 start=True, stop=True)
                if (dve_assist and st % 2 == 1) or st % 16 < NGP:
                    nc.vector.tensor_scalar(out=att[:], in0=atps[:], scalar1=0.0,
                                            scalar2=None, op0=OP.max)
                else:
                    nc.scalar.activation(out=att[:], in_=atps[:], func=AF.Relu)
                atts[(bh, lc, st)] = att

        def emit_av(bh, lc, avp, lb):
            vtb = tiles[bh][2]
            for st in range(NST):
                nc.tensor.matmul(avp[:, lb * 64:(lb + 1) * 64],
                                 lhsT=atts[(bh, lc, st)][:, lb * 128:(lb + 1) * 128],
                                 rhs=vtb[:, st * 64:(st + 1) * 64],
                                 start=(st == 0), stop=(st == NST - 1))
            if lb == 3:
                for st in range(NST):
                    atts.pop((bh, lc, st))

        def emit_avs_tail(bh, lc, avp, dve_assist=False):
            avs = outp.tile([128, 256], F32, tag="avs", name=f"avs{bh}{lc}")
            if dve_assist:
                nc.vector.tensor_scalar(out=avs[:], in0=avp[:], scalar1=0.0,
                                        scalar2=None, op0=OP.bypass)
            else:
                nc.scalar.activation(out=avs[:], in_=avp[:], func=AF.Copy)
            nc.scalar.dma_start(
                out=o[bh, lc * 512:(lc + 1) * 512, :].rearrange(
                    "(lb p) d -> p lb d", p=128),
                in_=avs[:].rearrange("p (lb d) -> p lb d", lb=4))

        units = [(bh, lc) for bh in range(BHC) for lc in range(NLC)]
        NU = len(units)
        avps = {}
        gts = {}
        NSEG = (NU - 1) * 4 + AV_D + 4
        for g in range(NSEG):
            w, seg = divmod(g, 4)
            if w < NU:
                bh, lc = units[w]
                if w == 0 and seg == 0:
                    phase_a()
                if seg == 0:
                    gts[w] = smallp.tile([128, 324], F32, tag="gt",
                                         name=f"gt{bh}{lc}")
                emit_r1_tile(bh, lc, seg, gts[w])
            zg = g - ZT_D
            if 0 <= zg < NU * 4:
                zw, zseg = divmod(zg, 4)
                pbh, plc = units[zw]
                emit_zt_evict(pbh, plc, zseg * 4, 4, dve_assist=(zw >= NU - 2))
            ag = g - AV_D
            if 0 <= ag < NU * 4:
                aw, aseg = divmod(ag, 4)
                qbh, qlc = units[aw]
                emit_av(qbh, qlc, avps[(qbh, qlc)], aseg)
                if aseg == 3:
                    emit_avs_tail(qbh, qlc, avps.pop((qbh, qlc)),
                                  dve_assist=(aw >= NU - 2))
            if w < NU and seg == 3:
                bh, lc = units[w]
                emit_r1_tail(bh, lc, gts.pop(w))
                avps[(bh, lc)] = psAV.tile([128, 256], F32, tag="av",
                                           name=f"av{bh}{lc}")
    nc.finalize()
    return nc


def _get_nc():
    global _nc
    if _nc is None:
        _nc = _build()
    return _nc


def _make_in_maps(queries, keys, values):
    # host-side prep: Q^T/8 [bh, E, L], K^T + (-1) row [bh, E+1, S] (f32),
    # V natural [bh, S, D] fp16, and the 9x9 tau-grid reciprocal table.
    qs = np.ascontiguousarray(
        queries.transpose(0, 2, 3, 1).reshape(B * H, E, L) * np.float32(0.125)
    ).astype(np.float32, copy=False)
    ks = keys.transpose(0, 2, 3, 1).reshape(B * H, E, S).astype(np.float32, copy=False)
    khs = np.concatenate(
        [ks, np.full((B * H, 1, S), -1.0, dtype=np.float32)], axis=1)
    khs = np.ascontiguousarray(khs)
    vs = np.ascontiguousarray(
        values.transpose(0, 2, 1, 3).reshape(B * H, S, D)).astype(np.float16)
    rec = np.empty(81, dtype=np.float32)
    for i in range(9):
        for j in range(9):
            rec[i * 9 + j] = 1e30 if i == 0 and j == 0 else 1.0 / (i + j)
    cst = np.tile(rec[None, :], (128, 1))
    return [
        {"qt": qs[c * BHC:(c + 1) * BHC], "kh": khs[c * BHC:(c + 1) * BHC],
         "vb": vs[c * BHC:(c + 1) * BHC], "cst": cst}
        for c in range(NCORES)
    ]


def _assemble(results):
    out = np.concatenate([results[c]["o"] for c in range(NCORES)], axis=0)  # [B*H, L, D]
    return np.ascontiguousarray(
        out.reshape(B, H, L, D).transpose(0, 2, 1, 3))  # [B, L, H, D]


def run_traced(queries, keys, values, **trace_kwargs):
    """Run with NTFF profiling; returns (output, BassKernelResults)."""
    from concourse.bass_utils import run_bass_kernel_spmd
    res = run_bass_kernel_spmd(_get_nc(), _make_in_maps(queries, keys, values),
                               core_ids=list(range(NCORES)), trace=True, **trace_kwargs)
    return _assemble(res.results), res


def kernel(queries, keys, values):
    from concourse.bass_utils import run_bass_kernel_spmd
    res = run_bass_kernel_spmd(_get_nc(), _make_in_maps(queries, keys, values),
                               core_ids=list(range(NCORES)))
    return _assemble(res.results)


# revision 10
# speedup vs baseline: 1.0577x; 1.0577x over previous
"""Sparsemax attention (B=2, H=16, L=S=2048, E=D=64, fp32) on 8 NeuronCores.

Strategy v5 (batch*head parallel, 4 (b,h) pairs per core):
  Three PE passes per unit (bh, 512-l-chunk), but the AV pass is *flipped*:
  A^T tiles (fp16, evicted by ACT/GPSIMD) are the stationary operand and V
  (fp16) is the moving operand, so AV costs 64 rows per matmul instead of
  512 -> PE drops from 164us to ~137us/core.

  Round 1 (per l-tile [128, S]): z = (Q K^T)/8 into PSUM; DVE top-8 per
    1024-half (exact; window schemes lose high-ranked support elements and
    blow the error gate).  tau is computed WITHOUT sorting the 16 candidates:
    with PA_i = prefix sums of the sorted h1-top8 and PB_j of h2-top8,
       tau = max_{i,j<=8} (PA_i + PB_j - 1) / (i+j)
    (every k<=16-prefix of the merged list is some (i,j) cell, and every other
    cell underestimates).  The 9x9 grid is built by GPSIMD TT-add/TT-mult on
    0-stride broadcast APs; DVE only does the two 8-wide prefix scans and one
    batched tensor_reduce(max) per unit.  DVE total ~168us/core = the wall.

  Round 2: z^T - tau via the 65th contraction row (khat row 64 = -1, qhat
    row 64 = tau, gathered by one SBUF->SBUF DMA per unit); Relu-evictions to
    fp16 att tiles split ACT(13)/GPSIMD(3) per unit -- DVE does none.

  AV: for lb in 0..3: accumulate over all 16 s-tiles into avp[:, lb*64:+64]
    ([128,256] f32 PSUM), lhsT = att[st][:, lb*128:+128] (fp16), rhs = vt[st]
    [128,64] fp16.  Output is [l, d] natural layout -> host just reshapes.

  Emission software-pipelines r1(u) / zt(u-ZT_D/4) / av(u-AV_D/4) at l-tile
  granularity as in v4.  All DMA on HWDGE queues (sync: tau gathers; scalar:
  bulk) -- the GPSIMD engine does evictions + grid work instead of SWDGE.
"""

import numpy as np

B, L, S, H, E, D = 2, 2048, 2048, 16, 64, 64
NCORES = 8
BHC = (B * H) // NCORES   # bh pairs per core = 4
NST = S // 128            # 16 s-tiles
NLC = L // 512            # 4 l-chunks

_nc = None


def _build():
    import os
    import concourse.bacc as bacc
    import concourse.mybir as mybir
    from concourse import tile

    F32 = mybir.dt.float32
    F32R = mybir.dt.float32r
    F16 = mybir.dt.float16
    AF = mybir.ActivationFunctionType
    OP = mybir.AluOpType
    AX = mybir.AxisListType

    nc = bacc.Bacc("TRN2", target_bir_lowering=False, debug=False)
    qt = nc.dram_tensor("qt", (BHC, E, L), F32R, kind="ExternalInput").ap()
    kh = nc.dram_tensor("kh", (BHC, E + 1, S), F32R, kind="ExternalInput").ap()
    vb = nc.dram_tensor("vb", (BHC, S, D), F16, kind="ExternalInput").ap()
    cst = nc.dram_tensor("cst", (128, 81), F32, kind="ExternalInput").ap()
    o = nc.dram_tensor("o", (BHC, L, D), F32, kind="ExternalOutput").ap()

    ZT_D = int(os.environ.get("ZT_D", "4"))
    AV_D = int(os.environ.get("AV_D", "8"))
    NGP = int(os.environ.get("NGP", "0"))     # DVE evicts per unit (of 16)

    with tile.TileContext(nc) as tc, \
         tc.tile_pool(name="const", bufs=1) as constp, \
         tc.tile_pool(name="big", bufs=4) as bigp, \
         tc.tile_pool(name="small", bufs=4) as smallp, \
         tc.tile_pool(name="att", bufs=44) as atp, \
         tc.tile_pool(name="outp", bufs=3) as outp, \
         tc.tile_pool(name="psA", bufs=2, space="PSUM") as psA, \
         tc.tile_pool(name="psAT", bufs=3, space="PSUM") as psAT, \
         tc.tile_pool(name="psAV", bufs=1, space="PSUM") as psAV:

        rec81 = constp.tile([128, 81], F32)

        tiles = {}

        def phase_a():
            # load order matters: HWDGE generates descriptors serially
            # (632ns/call), so K/Q chunks go first in unit-consumption order
            # (bh-major), then the grid constant, then V (needed only AV_D
            # segments in).
            for bh in range(BHC):
                qhat = bigp.tile([65, L], F32R, tag="qhat", name=f"qhat{bh}")
                khat = bigp.tile([65, S], F32R, tag="khat", name=f"khat{bh}")
                vtb = bigp.tile([128, NST * D], F16, tag="vtb", name=f"vtb{bh}")
                tiles[bh] = (qhat, khat, vtb)
            for bh in range(BHC):
                qhat, khat, _ = tiles[bh]
                if bh == 0:
                    # first Max needs khat s0:1024 + one qt l-block; sync queue
                    # stays otherwise clear for the latency-critical tau rows
                    nc.scalar.dma_start(out=khat[:, 0:512], in_=kh[0, :, 0:512])
                    nc.sync.dma_start(out=khat[:, 512:1024], in_=kh[0, :, 512:1024])
                    nc.scalar.dma_start(out=qhat[0:64, 0:512], in_=qt[0, :, 0:512])
                    nc.scalar.dma_start(out=khat[:, 1024:2048],
                                        in_=kh[0, :, 1024:2048])
                    nc.scalar.dma_start(out=qhat[0:64, 512:2048],
                                        in_=qt[0, :, 512:2048])
                    nc.scalar.dma_start(out=rec81[:], in_=cst[:])
                    continue
                for pos in (0, 1024):
                    nc.scalar.dma_start(out=khat[:, pos:pos + 1024],
                                        in_=kh[bh, :, pos:pos + 1024])
                    nc.scalar.dma_start(out=qhat[0:64, pos:pos + 1024],
                                        in_=qt[bh, :, pos:pos + 1024])
            for bh in range(BHC):
                nc.scalar.dma_start(
                    out=tiles[bh][2][:].rearrange("p (st d) -> p st d", st=NST),
                    in_=vb[bh].rearrange("(st p) d -> p st d", p=128))

        def emit_r1_tile(bh, lc, ii, gt):
            """Round 1 for one l-tile: z, top8 per half, prefix scans, grid."""
            qhat, khat, _ = tiles[bh]
            i = lc * 4 + ii
            cands = smallp.tile([128, 16], F32, tag="cands", name=f"cd{bh}{lc}{ii}")
            for c in range(2):
                ps = psA.tile([128, 1024], F32, tag="r1", name=f"ps{bh}{lc}{ii}{c}")
                for half in range(2):
                    nc.tensor.matmul(
                        ps[:, half * 512:(half + 1) * 512],
                        lhsT=qhat[0:64, i * 128:(i + 1) * 128],
                        rhs=khat[0:64, c * 1024 + half * 512:
                                 c * 1024 + (half + 1) * 512],
                        start=True, stop=True)
                nc.vector.max(out=cands[:, c * 8:(c + 1) * 8], in_=ps[:])
            pa = smallp.tile([128, 9], F32, tag="pa", name=f"pa{bh}{lc}{ii}")
            pb = smallp.tile([128, 9], F32, tag="pb", name=f"pb{bh}{lc}{ii}")
            nc.gpsimd.memset(pa[:, 0:1], -1.0)
            nc.gpsimd.memset(pb[:, 0:1], 0.0)
            nc.vector.tensor_tensor_scan(
                out=pa[:, 1:9], data0=cands[:, 0:8], data1=cands[:, 0:8],
                initial=-1.0, op0=OP.add, op1=OP.bypass)
            nc.vector.tensor_tensor_scan(
                out=pb[:, 1:9], data0=cands[:, 8:16], data1=cands[:, 8:16],
                initial=0.0, op0=OP.add, op1=OP.bypass)
            gadd = smallp.tile([128, 81], F32, tag="gadd", name=f"ga{bh}{lc}{ii}")
            nc.gpsimd.tensor_tensor(
                out=gadd[:].rearrange("p (i j) -> p i j", i=9),
                in0=pa[:].to_broadcast((128, 9, 9)),
                in1=pb[:].to_broadcast((128, 9, 9)).rearrange("p j i -> p i j"),
                op=OP.add)
            nc.gpsimd.tensor_tensor(
                out=gt[:, ii * 81:(ii + 1) * 81], in0=gadd[:], in1=rec81[:],
                op=OP.mult)

        def emit_r1_tail(bh, lc, gt):
            # batched tau for the 4 l-tiles + one SBUF->SBUF gather DMA into
            # qhat row 64 (the tau contraction row for round 2)
            tau4 = smallp.tile([128, 4], F32R, tag="tau4", name=f"t4{bh}{lc}")
            nc.vector.tensor_reduce(
                out=tau4[:], in_=gt[:].rearrange("p (lt c) -> p lt c", lt=4),
                axis=AX.X, op=OP.max)
            qhat = tiles[bh][0]
            for jj in range(4):
                nc.sync.dma_start(
                    out=qhat[64:65, lc * 512 + jj * 128:lc * 512 + (jj + 1) * 128],
                    in_=tau4[:, jj:jj + 1])

        atts = {}

        def emit_zt_evict(bh, lc, st0, n_st, dve_assist=False):
            """z^T - tau for s-tiles st0..+n_st, relu-evicted to fp16 SBUF.
            dve_assist: drain region -- DVE (idle there) takes every other
            eviction so the tail isn't serialized on ACT."""
            qhat, khat, _ = tiles[bh]
            for st in range(st0, st0 + n_st):
                atps = psAT.tile([128, 512], F32, tag="at", name=f"at{bh}{lc}{st}")
                att = atp.tile([128, 512], F16, tag="att", name=f"a{bh}{lc}{st}")
                nc.tensor.matmul(atps[:], lhsT=khat[:, st * 128:(st + 1) * 128],
                                 rhs=qhat[:, lc * 512:(lc + 1) * 512],
                                 start=True, stop=True)
                if (dve_assist and st % 2 == 1) or st % 16 < NGP:
                    nc.vector.tensor_scalar(out=att[:], in0=atps[:], scalar1=0.0,
                                            scalar2=None, op0=OP.max)
                else:
                    nc.scalar.activation(out=att[:], in_=atps[:], func=AF.Relu)
                atts[(bh, lc, st)] = att

        def emit_av(bh, lc, avp, lb):
            vtb = tiles[bh][2]
            for st in range(NST):
                nc.tensor.matmul(avp[:, lb * 64:(lb + 1) * 64],
                                 lhsT=atts[(bh, lc, st)][:, lb * 128:(lb + 1) * 128],
                                 rhs=vtb[:, st * 64:(st + 1) * 64],
                                 start=(st == 0), stop=(st == NST - 1))
            if lb == 3:
                for st in range(NST):
                    atts.pop((bh, lc, st))

        def emit_avs_tail(bh, lc, avp, dve_assist=False):
            avs = outp.tile([128, 256], F32, tag="avs", name=f"avs{bh}{lc}")
            if dve_assist:
                nc.vector.tensor_scalar(out=avs[:], in0=avp[:], scalar1=0.0,
                                        scalar2=None, op0=OP.bypass)
            else:
                nc.scalar.activation(out=avs[:], in_=avp[:], func=AF.Copy)
            nc.scalar.dma_start(
                out=o[bh, lc * 512:(lc + 1) * 512, :].rearrange(
                    "(lb p) d -> p lb d", p=128),
                in_=avs[:].rearrange("p (lb d) -> p lb d", lb=4))

        units = [(bh, lc) for bh in range(BHC) for lc in range(NLC)]
        NU = len(units)
        avps = {}
        gts = {}
        NSEG = (NU - 1) * 4 + AV_D + 4
        for g in range(NSEG):
            w, seg = divmod(g, 4)
            if w < NU:
                bh, lc = units[w]
                if w == 0 and seg == 0:
                    phase_a()
                if seg == 0:
                    gts[w] = smallp.tile([128, 324], F32, tag="gt",
                                         name=f"gt{bh}{lc}")
                emit_r1_tile(bh, lc, seg, gts[w])
            zg = g - ZT_D
            if 0 <= zg < NU * 4:
                zw, zseg = divmod(zg, 4)
                pbh, plc = units[zw]
                emit_zt_evict(pbh, plc, zseg * 4, 4, dve_assist=(zw >= NU - 2))
            ag = g - AV_D
            if 0 <= ag < NU * 4:
                aw, aseg = divmod(ag, 4)
                qbh, qlc = units[aw]
                emit_av(qbh, qlc, avps[(qbh, qlc)], aseg)
                if aseg == 3:
                    emit_avs_tail(qbh, qlc, avps.pop((qbh, qlc)),
                                  dve_assist=(aw >= NU - 2))
            if w < NU and seg == 3:
                bh, lc = units[w]
                emit_r1_tail(bh, lc, gts.pop(w))
                avps[(bh, lc)] = psAV.tile([128, 256], F32, tag="av",
                                           name=f"av{bh}{lc}")
    nc.finalize()
    return nc


def _get_nc():
    global _nc
    if _nc is None:
        _nc = _build()
    return _nc


def _make_in_maps(queries, keys, values):
    # host-side prep: Q^T/8 [bh, E, L], K^T + (-1) row [bh, E+1, S] (f32),
    # V natural [bh, S, D] fp16, and the 9x9 tau-grid reciprocal table.
    qs = np.ascontiguousarray(
        queries.transpose(0, 2, 3, 1).reshape(B * H, E, L) * np.float32(0.125)
    ).astype(np.float32, copy=False)
    ks = keys.transpose(0, 2, 3, 1).reshape(B * H, E, S).astype(np.float32, copy=False)
    khs = np.concatenate(
        [ks, np.full((B * H, 1, S), -1.0, dtype=np.float32)], axis=1)
    khs = np.ascontiguousarray(khs)
    vs = np.ascontiguousarray(
        values.transpose(0, 2, 1, 3).reshape(B * H, S, D)).astype(np.float16)
    rec = np.empty(81, dtype=np.float32)
    for i in range(9):
        for j in range(9):
            rec[i * 9 + j] = 1e30 if i == 0 and j == 0 else 1.0 / (i + j)
    cst = np.tile(rec[None, :], (128, 1))
    return [
        {"qt": qs[c * BHC:(c + 1) * BHC], "kh": khs[c * BHC:(c + 1) * BHC],
         "vb": vs[c * BHC:(c + 1) * BHC], "cst": cst}
        for c in range(NCORES)
    ]


def _assemble(results):
    out = np.concatenate([results[c]["o"] for c in range(NCORES)], axis=0)  # [B*H, L, D]
    return np.ascontiguousarray(
        out.reshape(B, H, L, D).transpose(0, 2, 1, 3))  # [B, L, H, D]


def run_traced(queries, keys, values, **trace_kwargs):
    """Run with NTFF profiling; returns (output, BassKernelResults)."""
    from concourse.bass_utils import run_bass_kernel_spmd
    res = run_bass_kernel_spmd(_get_nc(), _make_in_maps(queries, keys, values),
                               core_ids=list(range(NCORES)), trace=True, **trace_kwargs)
    return _assemble(res.results), res


def kernel(queries, keys, values):
    from concourse.bass_utils import run_bass_kernel_spmd
    res = run_bass_kernel_spmd(_get_nc(), _make_in_maps(queries, keys, values),
                               core_ids=list(range(NCORES)))
    return _assemble(res.results)


# revision 11
# speedup vs baseline: 1.0768x; 1.0181x over previous
"""Sparsemax attention (B=2, H=16, L=S=2048, E=D=64, fp32) on 8 NeuronCores.

Strategy v5 (batch*head parallel, 4 (b,h) pairs per core):
  Three PE passes per unit (bh, 512-l-chunk), but the AV pass is *flipped*:
  A^T tiles (fp16, evicted by ACT/GPSIMD) are the stationary operand and V
  (fp16) is the moving operand, so AV costs 64 rows per matmul instead of
  512 -> PE drops from 164us to ~137us/core.

  Round 1 (per l-tile [128, S]): z = (Q K^T)/8 into PSUM; DVE top-8 per
    1024-half (exact; window schemes lose high-ranked support elements and
    blow the error gate).  tau is computed WITHOUT sorting the 16 candidates:
    with PA_i = prefix sums of the sorted h1-top8 and PB_j of h2-top8,
       tau = max_{i,j<=8} (PA_i + PB_j - 1) / (i+j)
    (every k<=16-prefix of the merged list is some (i,j) cell, and every other
    cell underestimates).  The 9x9 grid is built by GPSIMD TT-add/TT-mult on
    0-stride broadcast APs; DVE only does the two 8-wide prefix scans and one
    batched tensor_reduce(max) per unit.  DVE total ~168us/core = the wall.

  Round 2: z^T - tau via the 65th contraction row (khat row 64 = -1, qhat
    row 64 = tau, gathered by one SBUF->SBUF DMA per unit); Relu-evictions to
    fp16 att tiles split ACT(13)/GPSIMD(3) per unit -- DVE does none.

  AV: for lb in 0..3: accumulate over all 16 s-tiles into avp[:, lb*64:+64]
    ([128,256] f32 PSUM), lhsT = att[st][:, lb*128:+128] (fp16), rhs = vt[st]
    [128,64] fp16.  Output is [l, d] natural layout -> host just reshapes.

  Emission software-pipelines r1(u) / zt(u-ZT_D/4) / av(u-AV_D/4) at l-tile
  granularity as in v4.  All DMA on HWDGE queues (sync: tau gathers; scalar:
  bulk) -- the GPSIMD engine does evictions + grid work instead of SWDGE.
"""

import numpy as np

B, L, S, H, E, D = 2, 2048, 2048, 16, 64, 64
NCORES = 8
BHC = (B * H) // NCORES   # bh pairs per core = 4
NST = S // 128            # 16 s-tiles
NLC = L // 512            # 4 l-chunks

_nc = None


def _build():
    import os
    import concourse.bacc as bacc
    import concourse.mybir as mybir
    from concourse import tile

    F32 = mybir.dt.float32
    F32R = mybir.dt.float32r
    F16 = mybir.dt.float16
    AF = mybir.ActivationFunctionType
    OP = mybir.AluOpType
    AX = mybir.AxisListType

    nc = bacc.Bacc("TRN2", target_bir_lowering=False, debug=False)
    qt = nc.dram_tensor("qt", (BHC, E, L), F32R, kind="ExternalInput").ap()
    kh = nc.dram_tensor("kh", (BHC, E + 1, S), F32R, kind="ExternalInput").ap()
    vb = nc.dram_tensor("vb", (BHC, S, D), F16, kind="ExternalInput").ap()
    cst = nc.dram_tensor("cst", (128, 81), F32, kind="ExternalInput").ap()
    o = nc.dram_tensor("o", (BHC, L, D), F32, kind="ExternalOutput").ap()

    ZT_D = int(os.environ.get("ZT_D", "4"))
    AV_D = int(os.environ.get("AV_D", "8"))
    NGP = int(os.environ.get("NGP", "0"))     # DVE evicts per unit (of 16)

    with tile.TileContext(nc) as tc, \
         tc.tile_pool(name="const", bufs=1) as constp, \
         tc.tile_pool(name="big", bufs=4) as bigp, \
         tc.tile_pool(name="small", bufs=4) as smallp, \
         tc.tile_pool(name="att", bufs=44) as atp, \
         tc.tile_pool(name="outp", bufs=3) as outp, \
         tc.tile_pool(name="psA", bufs=2, space="PSUM") as psA, \
         tc.tile_pool(name="psAT", bufs=3, space="PSUM") as psAT, \
         tc.tile_pool(name="psAV", bufs=1, space="PSUM") as psAV:

        rec81 = constp.tile([128, 81], F32)

        tiles = {}

        def phase_a():
            # load order matters: HWDGE generates descriptors serially
            # (632ns/call), so K/Q chunks go first in unit-consumption order
            # (bh-major), then the grid constant, then V (needed only AV_D
            # segments in).
            for bh in range(BHC):
                qhat = bigp.tile([65, L], F32R, tag="qhat", name=f"qhat{bh}")
                khat = bigp.tile([65, S], F32R, tag="khat", name=f"khat{bh}")
                vtb = bigp.tile([128, NST * D], F16, tag="vtb", name=f"vtb{bh}")
                tiles[bh] = (qhat, khat, vtb)
            for bh in range(BHC):
                qhat, khat, _ = tiles[bh]
                qq = nc.sync if bh == 0 else nc.scalar
                chunks = (512, 512, 1024) if bh == 0 else (1024, 1024)
                pos = 0
                for w in chunks:
                    nc.scalar.dma_start(out=khat[:, pos:pos + w],
                                        in_=kh[bh, :, pos:pos + w])
                    qq.dma_start(out=qhat[0:64, pos:pos + w],
                                 in_=qt[bh, :, pos:pos + w])
                    pos += w
                if bh == 0:
                    nc.sync.dma_start(out=rec81[:], in_=cst[:])
            for bh in range(BHC):
                nc.scalar.dma_start(
                    out=tiles[bh][2][:].rearrange("p (st d) -> p st d", st=NST),
                    in_=vb[bh].rearrange("(st p) d -> p st d", p=128))

        def emit_r1_tile(bh, lc, ii, gt):
            """Round 1 for one l-tile: z, top8 per half, prefix scans, grid."""
            qhat, khat, _ = tiles[bh]
            i = lc * 4 + ii
            cands = smallp.tile([128, 16], F32, tag="cands", name=f"cd{bh}{lc}{ii}")
            for c in range(2):
                ps = psA.tile([128, 1024], F32, tag="r1", name=f"ps{bh}{lc}{ii}{c}")
                for half in range(2):
                    nc.tensor.matmul(
                        ps[:, half * 512:(half + 1) * 512],
                        lhsT=qhat[0:64, i * 128:(i + 1) * 128],
                        rhs=khat[0:64, c * 1024 + half * 512:
                                 c * 1024 + (half + 1) * 512],
                        start=True, stop=True)
                nc.vector.max(out=cands[:, c * 8:(c + 1) * 8], in_=ps[:])
            pa = smallp.tile([128, 9], F32, tag="pa", name=f"pa{bh}{lc}{ii}")
            pb = smallp.tile([128, 9], F32, tag="pb", name=f"pb{bh}{lc}{ii}")
            nc.gpsimd.memset(pa[:, 0:1], -1.0)
            nc.gpsimd.memset(pb[:, 0:1], 0.0)
            nc.vector.tensor_tensor_scan(
                out=pa[:, 1:9], data0=cands[:, 0:8], data1=cands[:, 0:8],
                initial=-1.0, op0=OP.add, op1=OP.bypass)
            nc.vector.tensor_tensor_scan(
                out=pb[:, 1:9], data0=cands[:, 8:16], data1=cands[:, 8:16],
                initial=0.0, op0=OP.add, op1=OP.bypass)
            gadd = smallp.tile([128, 81], F32, tag="gadd", name=f"ga{bh}{lc}{ii}")
            nc.gpsimd.tensor_tensor(
                out=gadd[:].rearrange("p (i j) -> p i j", i=9),
                in0=pa[:].to_broadcast((128, 9, 9)),
                in1=pb[:].to_broadcast((128, 9, 9)).rearrange("p j i -> p i j"),
                op=OP.add)
            nc.gpsimd.tensor_tensor(
                out=gt[:, ii * 81:(ii + 1) * 81], in0=gadd[:], in1=rec81[:],
                op=OP.mult)

        def emit_r1_tail(bh, lc, gt):
            # batched tau for the 4 l-tiles + one SBUF->SBUF gather DMA into
            # qhat row 64 (the tau contraction row for round 2)
            tau4 = smallp.tile([128, 4], F32R, tag="tau4", name=f"t4{bh}{lc}")
            nc.vector.tensor_reduce(
                out=tau4[:], in_=gt[:].rearrange("p (lt c) -> p lt c", lt=4),
                axis=AX.X, op=OP.max)
            qhat = tiles[bh][0]
            for jj in range(4):
                nc.sync.dma_start(
                    out=qhat[64:65, lc * 512 + jj * 128:lc * 512 + (jj + 1) * 128],
                    in_=tau4[:, jj:jj + 1])

        atts = {}

        def emit_zt_evict(bh, lc, st0, n_st, dve_assist=False):
            """z^T - tau for s-tiles st0..+n_st, relu-evicted to fp16 SBUF.
            dve_assist: drain region -- DVE (idle there) takes every other
            eviction so the tail isn't serialized on ACT."""
            qhat, khat, _ = tiles[bh]
            for st in range(st0, st0 + n_st):
                atps = psAT.tile([128, 512], F32, tag="at", name=f"at{bh}{lc}{st}")
                att = atp.tile([128, 512], F16, tag="att", name=f"a{bh}{lc}{st}")
                nc.tensor.matmul(atps[:], lhsT=khat[:, st * 128:(st + 1) * 128],
                                 rhs=qhat[:, lc * 512:(lc + 1) * 512],
                                 start=True, stop=True)
                if (dve_assist and st % 2 == 1) or st % 16 < NGP:
                    nc.vector.tensor_scalar(out=att[:], in0=atps[:], scalar1=0.0,
                                            scalar2=None, op0=OP.max)
                else:
                    nc.scalar.activation(out=att[:], in_=atps[:], func=AF.Relu)
                atts[(bh, lc, st)] = att

        def emit_av(bh, lc, avp, lb):
            vtb = tiles[bh][2]
            for st in range(NST):
                nc.tensor.matmul(avp[:, lb * 64:(lb + 1) * 64],
                                 lhsT=atts[(bh, lc, st)][:, lb * 128:(lb + 1) * 128],
                                 rhs=vtb[:, st * 64:(st + 1) * 64],
                                 start=(st == 0), stop=(st == NST - 1))
            if lb == 3:
                for st in range(NST):
                    atts.pop((bh, lc, st))

        def emit_avs_tail(bh, lc, avp, dve_assist=False):
            avs = outp.tile([128, 256], F32, tag="avs", name=f"avs{bh}{lc}")
            if dve_assist:
                nc.vector.tensor_scalar(out=avs[:], in0=avp[:], scalar1=0.0,
                                        scalar2=None, op0=OP.bypass)
            else:
                nc.scalar.activation(out=avs[:], in_=avp[:], func=AF.Copy)
            nc.scalar.dma_start(
                out=o[bh, lc * 512:(lc + 1) * 512, :].rearrange(
                    "(lb p) d -> p lb d", p=128),
                in_=avs[:].rearrange("p (lb d) -> p lb d", lb=4))

        units = [(bh, lc) for bh in range(BHC) for lc in range(NLC)]
        NU = len(units)
        avps = {}
        gts = {}
        NSEG = (NU - 1) * 4 + AV_D + 4
        for g in range(NSEG):
            w, seg = divmod(g, 4)
            if w < NU:
                bh, lc = units[w]
                if w == 0 and seg == 0:
                    phase_a()
                if seg == 0:
                    gts[w] = smallp.tile([128, 324], F32, tag="gt",
                                         name=f"gt{bh}{lc}")
                emit_r1_tile(bh, lc, seg, gts[w])
            zg = g - ZT_D
            if 0 <= zg < NU * 4:
                zw, zseg = divmod(zg, 4)
                pbh, plc = units[zw]
                emit_zt_evict(pbh, plc, zseg * 4, 4, dve_assist=(zw >= NU - 2))
            ag = g - AV_D
            if 0 <= ag < NU * 4:
                aw, aseg = divmod(ag, 4)
                qbh, qlc = units[aw]
                emit_av(qbh, qlc, avps[(qbh, qlc)], aseg)
                if aseg == 3:
                    emit_avs_tail(qbh, qlc, avps.pop((qbh, qlc)),
                                  dve_assist=(aw >= NU - 2))
            if w < NU and seg == 3:
                bh, lc = units[w]
                emit_r1_tail(bh, lc, gts.pop(w))
                avps[(bh, lc)] = psAV.tile([128, 256], F32, tag="av",
                                           name=f"av{bh}{lc}")
    nc.finalize()
    return nc


def _get_nc():
    global _nc
    if _nc is None:
        _nc = _build()
    return _nc


def _make_in_maps(queries, keys, values):
    # host-side prep: Q^T/8 [bh, E, L], K^T + (-1) row [bh, E+1, S] (f32),
    # V natural [bh, S, D] fp16, and the 9x9 tau-grid reciprocal table.
    qs = np.ascontiguousarray(
        queries.transpose(0, 2, 3, 1).reshape(B * H, E, L) * np.float32(0.125)
    ).astype(np.float32, copy=False)
    ks = keys.transpose(0, 2, 3, 1).reshape(B * H, E, S).astype(np.float32, copy=False)
    khs = np.concatenate(
        [ks, np.full((B * H, 1, S), -1.0, dtype=np.float32)], axis=1)
    khs = np.ascontiguousarray(khs)
    vs = np.ascontiguousarray(
        values.transpose(0, 2, 1, 3).reshape(B * H, S, D)).astype(np.float16)
    rec = np.empty(81, dtype=np.float32)
    for i in range(9):
        for j in range(9):
            rec[i * 9 + j] = 1e30 if i == 0 and j == 0 else 1.0 / (i + j)
    cst = np.tile(rec[None, :], (128, 1))
    return [
        {"qt": qs[c * BHC:(c + 1) * BHC], "kh": khs[c * BHC:(c + 1) * BHC],
         "vb": vs[c * BHC:(c + 1) * BHC], "cst": cst}
        for c in range(NCORES)
    ]


def _assemble(results):
    out = np.concatenate([results[c]["o"] for c in range(NCORES)], axis=0)  # [B*H, L, D]
    return np.ascontiguousarray(
        out.reshape(B, H, L, D).transpose(0, 2, 1, 3))  # [B, L, H, D]


def run_traced(queries, keys, values, **trace_kwargs):
    """Run with NTFF profiling; returns (output, BassKernelResults)."""
    from concourse.bass_utils import run_bass_kernel_spmd
    res = run_bass_kernel_spmd(_get_nc(), _make_in_maps(queries, keys, values),
                               core_ids=list(range(NCORES)), trace=True, **trace_kwargs)
    return _assemble(res.results), res


def kernel(queries, keys, values):
    from concourse.bass_utils import run_bass_kernel_spmd
    res = run_bass_kernel_spmd(_get_nc(), _make_in_maps(queries, keys, values),
                               core_ids=list(range(NCORES)))
    return _assemble(res.results)
